# revision 19
# baseline (speedup 1.0000x reference)
"""Distributed Trainium2 kernel for nn_Attention_31370441130243.

Full-input / full-output attention layer, sharded internally over the
8 NeuronCores as (batch=2) x (head-group=4): core c handles batch c//4
and heads [4*(c%4), 4*(c%4)+4).  Each core computes its QKV projections,
per-head RMSNorm + RoPE, non-causal SDPA and a partial output projection
(its Wout column block); the host sums the 4 partials per batch.

Layout strategy per core (S=2048 seq, D=2048 model, 4 local heads, C=128):
  - Q,K projections run with transposed-x stationary tiles (lhsT = xT
    block, rhs = W.T block) producing q,k in [s, c] layout where RMSNorm
    (free-dim reduce) and RoPE (free-dim half-rotation) are cheap;
    post-processing is vectorized across all 4 heads ([128, 512] tiles).
  - q,k are PE-transposed into qT,kT [c, s] for the attention matmuls
    (4 transposes share one PSUM bank, one wide copy out).
  - Scores are computed transposed (scT = kT_blk.T @ qT = [t, s]) so the
    PV matmul needs no P transpose: out.T = v_blk.T @ exp(scT).  Scores
    for two t-chunks share a [128, 1024] PSUM tile -> one Exp per pair.
  - exp() is written as bf16; the softmax denominator is accumulated on
    the vector engine (running adds of the e2 halves), then a single
    ones[128x128] matmul per (head, s-chunk) both partition-reduces and
    broadcasts it into PSUM; normalization is reciprocal + multiply on
    the vector engine.  (This removes the 16-per-head ones-column
    denominator matmuls the PE used to execute.)
  - Matmuls use float32r (full-rate fp32 on the PE at N>=256); the PV
    matmul streams the bf16 exp tiles.
  - DMA issue order is tuned so the PE can start within ~2us: first Wq/Wk
    slice, then the g=0/g=1 x blocks, then remaining weight slices, then
    rope tables.
"""

import math
import sys

import ml_dtypes
import numpy as np

for _p in ("/opt/trn_rl_repo",):
    if _p not in sys.path:
        sys.path.append(_p)

import bass_rust

import concourse.bass as bass
import concourse.tile as tile
from concourse import mybir
from concourse.bass_utils import run_bass_kernel_spmd
from concourse.masks import make_identity
from concourse.vector_clock import ScopedClock

S, B, D = 2048, 2, 2048
H, C = 16, 128
HL = 4                 # heads per core
M = HL * C             # local qkv rows (512)
EPS = 1e-6
NCORES = 8
INV_SQRT_C = 1.0 / math.sqrt(C)

f32 = mybir.dt.float32
f32r = mybir.dt.float32r
bf16 = mybir.dt.bfloat16
Act = mybir.ActivationFunctionType


# ---------------------------------------------------------------------------
# This container's walrus accepts at most one sync-wait command per
# instruction; the stock TileContext exit drain carries one wait per
# outstanding proc.  Split them onto single-wait NoOps.
def _split_drain_and_barrier(self, tick_clock, wait_clock):
    nc = self.nc
    probe = nc.sync.nop(nofuse=True, hint="tile_exit_waits")
    wait_clock.add_sem_waits(probe.ins, ScopedClock({None: tick_clock.global_clock}))
    si = probe.ins.sync_info
    if si is not None and si.on_wait is not None and len(si.on_wait) > 1:
        waits = list(si.on_wait)
        si.on_wait = [waits[0]]
        for w in waits[1:]:
            n2 = nc.sync.nop(nofuse=True, hint="tile_exit_waits")
            n2.ins.sync_info = bass_rust.SyncInfo(on_wait=[w], on_update=[])
    nc.sync.drain(fusable=False)
    nc.all_engine_barrier()
    popped = nc._tile_sem_poison_stack.pop()
    assert popped is self._sem_poison
    nc.clear_and_free_semaphores(list(self.sems.allocated().values()))
    nc.all_engine_barrier()


tile.TileContext._drain_and_barrier = _split_drain_and_barrier


def _split_multi_waits(nc):
    """Walrus here accepts one sync-wait per instruction; hoist extras onto
    single-wait NoOps on the same engine immediately before the instruction."""
    for f in nc.m.functions:
        for bb in f.blocks:
            out = []
            changed = False
            for inst in bb.instructions:
                si = inst.sync_info
                if si is not None and si.on_wait is not None and len(si.on_wait) > 1:
                    waits = list(si.on_wait)
                    si.on_wait = [waits[-1]]
                    for w in waits[:-1]:
                        nop = mybir.InstNoOp(
                            name=f"I-{nc.next_id()}",
                            engine=inst.engine,
                            sync_info=mybir.SyncInfo(on_wait=[w], on_update=[]),
                            bass_nofuse=True,
                        )
                        out.append(nop)
                    changed = True
                out.append(inst)
            if changed:
                bb.instructions[:] = out


def _bcast_heads(ap_2d, heads):
    """View a [128, C] AP as [128, heads, C] with a zero-stride head dim."""
    return bass.AP(
        tensor=ap_2d.tensor,
        offset=ap_2d.offset,
        ap=[ap_2d.ap[0], [0, heads], ap_2d.ap[1]],
    )


def build_core_kernel(s=S, d=D, split_waits=True):
    """One core's kernel: partial attention output for 4 heads of 1 batch."""
    st, dt, nsc, tt = s // 128, d // 128, s // 512, s // 128
    nc = bass.Bass()

    xT = nc.declare_dram_parameter("xT", [d, s], bf16, isOutput=False)
    wq = nc.declare_dram_parameter("wq", [d, M], bf16, isOutput=False)
    wk = nc.declare_dram_parameter("wk", [d, M], bf16, isOutput=False)
    wv = nc.declare_dram_parameter("wv", [d, M], bf16, isOutput=False)
    wout = nc.declare_dram_parameter("wout", [M, d], bf16, isOutput=False)
    cosf = nc.declare_dram_parameter("cosf", [s, C], f32, isOutput=False)
    ssinf = nc.declare_dram_parameter("ssinf", [s, C], f32, isOutput=False)
    qs = nc.declare_dram_parameter("qs", [C], f32, isOutput=False)
    ks = nc.declare_dram_parameter("ks", [C], f32, isOutput=False)
    out = nc.declare_dram_parameter("out", [s, d], f32, isOutput=True)

    xT_r = xT.rearrange("(n p) t -> p n t", p=128)

    with tile.TileContext(nc) as tc:
        with (
            tc.tile_pool(name="const", bufs=1) as constp,
            tc.tile_pool(name="qkt", bufs=1) as qktp,
        ):
            qT = qktp.tile([128, HL, s], bf16, name="qT")
            kT = qktp.tile([128, HL, s], bf16, name="kT")

            # ---- phase 1: Q,K proj + rmsnorm + rope + transpose ----
            with tc.tile_pool(name="rope", bufs=1) as ropep:
                cos_t = ropep.tile([128, st, C], f32, name="cos_t")
                ssin_t = ropep.tile([128, st, C], f32, name="ssin_t")
                qs_bc = constp.tile([128, C], f32, name="qs_bc")
                ks_bc = constp.tile([128, C], f32, name="ks_bc")

                with (
                    tc.tile_pool(name="wqk", bufs=1) as wqkp,
                    tc.tile_pool(name="ph1", bufs=2) as ph1,
                    tc.tile_pool(name="accps", bufs=6, space="PSUM") as accps,
                    tc.tile_pool(name="tps", bufs=2, space="PSUM") as tps,
                ):
                    # DMA issue order matters: the sync HWDGE ring is FIFO,
                    # so put what the PE needs first at the head.
                    wq_sb = wqkp.tile([128, dt, M], bf16, name="wq_sb")
                    wk_sb = wqkp.tile([128, dt, M], bf16, name="wk_sb")
                    wq_r = wq.rearrange("(n p) m -> p n m", p=128)
                    wk_r = wk.rearrange("(n p) m -> p n m", p=128)

                    xr_pending = {}

                    def load_xr(g):
                        xrows = []
                        for dh in range(4):
                            xr = ph1.tile(
                                [128, dt // 4, 256], bf16, name="xr",
                                tag="xr", bufs=8,
                            )
                            nc.sync.dma_start(
                                out=xr,
                                in_=xT_r[
                                    :, dh * (dt // 4) : (dh + 1) * (dt // 4),
                                    g * 256 : (g + 1) * 256,
                                ],
                            )
                            xrows.append(xr)
                        xr_pending[g] = xrows

                    nc.sync.dma_start(out=wq_sb[:, 0, :], in_=wq_r[:, 0, :])
                    nc.sync.dma_start(out=wk_sb[:, 0, :], in_=wk_r[:, 0, :])
                    load_xr(0)
                    for n in range(1, 8):
                        nc.sync.dma_start(out=wq_sb[:, n, :], in_=wq_r[:, n, :])
                        nc.sync.dma_start(out=wk_sb[:, n, :], in_=wk_r[:, n, :])
                    load_xr(1)
                    for n in range(8, dt):
                        nc.sync.dma_start(out=wq_sb[:, n, :], in_=wq_r[:, n, :])
                        nc.sync.dma_start(out=wk_sb[:, n, :], in_=wk_r[:, n, :])

                    # scales, rope tables (needed only by post-processing,
                    # which trails the matmul stream by most of a g-group)
                    for w_bc, w_dram in ((qs_bc, qs), (ks_bc, ks)):
                        src = bass.AP(
                            tensor=w_dram.ap().tensor, offset=0,
                            ap=[[0, 128], [1, C]],
                        )
                        nc.sync.dma_start(out=w_bc, in_=src)
                    nc.sync.dma_start(
                        out=cos_t, in_=cosf.rearrange("(n p) c -> p n c", p=128)
                    )
                    nc.sync.dma_start(
                        out=ssin_t, in_=ssinf.rearrange("(n p) c -> p n c", p=128)
                    )

                    # constants: identity + all-ones (f32 build, f32r copies;
                    # memset rejects f32r operands in this walrus), eps,
                    # rotated scale copies for the rope sin term.
                    scratch_f = constp.tile([128, 128], f32, name="scratch_f")
                    make_identity(nc, scratch_f)
                    ident = constp.tile([128, 128], bf16, name="ident")
                    nc.vector.tensor_copy(out=ident, in_=scratch_f)
                    ones_f = constp.tile([128, 128], f32, name="ones_f")
                    nc.vector.memset(ones_f, 1.0)
                    ones_bf = constp.tile([128, 128], bf16, name="ones_bf")
                    nc.vector.tensor_copy(out=ones_bf, in_=ones_f)
                    eps_t = constp.tile([128, 1], f32, name="eps_t")
                    nc.vector.memset(eps_t, EPS)

                    qs_rot = constp.tile([128, C], f32, name="qs_rot")
                    ks_rot = constp.tile([128, C], f32, name="ks_rot")
                    for w_rot, w_bc in ((qs_rot, qs_bc), (ks_rot, ks_bc)):
                        nc.gpsimd.tensor_copy(
                            out=w_rot[:, 0 : C // 2], in_=w_bc[:, C // 2 : C]
                        )
                        nc.gpsimd.tensor_copy(
                            out=w_rot[:, C // 2 : C], in_=w_bc[:, 0 : C // 2]
                        )

                    for g in range(st // 2):
                        xrows = xr_pending.pop(g)
                        pacc = []
                        for jj in range(2):
                            pq = accps.tile([128, M], f32, name="pq", tag="acc")
                            pk = accps.tile([128, M], f32, name="pk", tag="acc")
                            pacc.append((pq, pk))
                        for n in range(dt):
                            for jj in range(2):
                                xsl = xrows[n // (dt // 4)][
                                    :, n % (dt // 4), jj * 128 : (jj + 1) * 128
                                ]
                                nc.tensor.matmul(
                                    pacc[jj][0], lhsT=xsl, rhs=wq_sb[:, n, :],
                                    start=(n == 0), stop=(n == dt - 1),
                                )
                                nc.tensor.matmul(
                                    pacc[jj][1], lhsT=xsl, rhs=wk_sb[:, n, :],
                                    start=(n == 0), stop=(n == dt - 1),
                                )
                        if g + 2 < st // 2:
                            load_xr(g + 2)
                        # free the psum accumulators as fast as possible: all
                        # four copies first, then the per-tile post-processing
                        xsbs = {}
                        for jj in range(2):
                            for qk in range(2):
                                xsb = ph1.tile([128, M], f32, name="xsb", bufs=5)
                                nc.scalar.copy(out=xsb, in_=pacc[jj][qk])
                                xsbs[(jj, qk)] = xsb
                        for jj in range(2):
                            j = g * 2 + jj
                            for qk, (w_bc, w_rot, dstT) in enumerate(
                                ((qs_bc, qs_rot, qT), (ks_bc, ks_rot, kT))
                            ):
                                xsb = xsbs[(jj, qk)]
                                # rmsnorm scale 1/rms from raw q (all heads):
                                # wide square + per-head free reduce
                                sqw = ph1.tile([128, M], f32, name="sqw", bufs=2)
                                nc.scalar.activation(
                                    out=sqw, in_=xsb, func=Act.Square
                                )
                                ssq4 = ph1.tile([128, HL, 1], f32, name="ssq4", bufs=3)
                                nc.vector.tensor_reduce(
                                    out=ssq4,
                                    in_=sqw.rearrange("p (a c) -> p a c", a=HL),
                                    op=mybir.AluOpType.add,
                                    axis=mybir.AxisListType.X,
                                )
                                rms4 = ph1.tile([128, HL], f32, name="rms4", bufs=3)
                                nc.scalar.activation(
                                    out=rms4,
                                    in_=ssq4.rearrange("p a one -> p (a one)"),
                                    func=Act.Sqrt, scale=1.0 / C, bias=eps_t,
                                )
                                r4 = ph1.tile([128, HL], f32, name="r4", bufs=3)
                                nc.vector.reciprocal(out=r4, in_=rms4)
                                # per-j rope tables folded with the channel
                                # scale (gpsimd, off the critical engines)
                                cw = ph1.tile([128, C], f32, name="cw", bufs=2)
                                nc.gpsimd.tensor_mul(
                                    out=cw, in0=cos_t[:, j, :], in1=w_bc
                                )
                                sw = ph1.tile([128, C], f32, name="sw", bufs=2)
                                nc.gpsimd.tensor_mul(
                                    out=sw, in0=ssin_t[:, j, :], in1=w_rot
                                )
                                # rotate_half into sh (wide, 3-d strided)
                                sh = ph1.tile([128, HL, C], f32, name="sh", bufs=3)
                                xsb3 = xsb.rearrange("p (a c) -> p a c", a=HL)
                                nc.vector.tensor_copy(
                                    out=sh[:, :, 0 : C // 2],
                                    in_=xsb3[:, :, C // 2 : C],
                                )
                                nc.vector.tensor_copy(
                                    out=sh[:, :, C // 2 : C],
                                    in_=xsb3[:, :, 0 : C // 2],
                                )
                                t1 = ph1.tile([128, HL, C], f32, name="t1", bufs=3)
                                nc.vector.tensor_mul(
                                    out=t1, in0=xsb3, in1=_bcast_heads(cw, HL)
                                )
                                nc.vector.tensor_mul(
                                    out=sh, in0=sh, in1=_bcast_heads(sw, HL)
                                )
                                nc.vector.tensor_add(out=t1, in0=t1, in1=sh)
                                # 1/rms per head via scalar-engine copy-with-
                                # scale (casts to bf16), then PE transpose
                                xrot = ph1.tile([128, M], bf16, name="xrot", bufs=3)
                                for h in range(HL):
                                    nc.scalar.activation(
                                        out=xrot[:, h * C : (h + 1) * C],
                                        in_=t1[:, h, :],
                                        func=Act.Copy,
                                        scale=r4[:, h : h + 1],
                                    )
                                pt4 = tps.tile([128, M], bf16, name="pt4")
                                for h in range(HL):
                                    nc.tensor.transpose(
                                        pt4[:, h * C : (h + 1) * C],
                                        xrot[:, h * C : (h + 1) * C],
                                        ident,
                                    )
                                nc.scalar.copy(
                                    out=dstT[:, :, j * 128 : (j + 1) * 128],
                                    in_=pt4.rearrange("p (a c) -> p a c", a=HL),
                                )

            # ---- phase 2: V projection ----
            with tc.tile_pool(name="vpool", bufs=1) as vpool:
                v_sb = vpool.tile([128, tt, M], bf16, name="v_sb")
                with (
                    tc.tile_pool(name="wvp", bufs=1) as wvp,
                    tc.tile_pool(name="ph2", bufs=2) as ph2,
                    tc.tile_pool(name="vps", bufs=4, space="PSUM") as vps,
                ):
                    wv_sb = wvp.tile([128, dt, M], bf16, name="wv_sb")
                    wv_r = wv.rearrange("(n p) m -> p n m", p=128)

                    x2_pending = {}

                    def load_x2(g):
                        xr2s = []
                        for dh in range(4):
                            x2 = ph2.tile(
                                [128, dt // 4, 256], bf16, name="x2",
                                tag="x2", bufs=8,
                            )
                            nc.sync.dma_start(
                                out=x2,
                                in_=xT_r[
                                    :, dh * (dt // 4) : (dh + 1) * (dt // 4),
                                    g * 256 : (g + 1) * 256,
                                ],
                            )
                            xr2s.append(x2)
                        x2_pending[g] = xr2s

                    nc.sync.dma_start(out=wv_sb[:, 0, :], in_=wv_r[:, 0, :])
                    load_x2(0)
                    for n in range(1, 8):
                        nc.sync.dma_start(out=wv_sb[:, n, :], in_=wv_r[:, n, :])
                    load_x2(1)
                    for n in range(8, dt):
                        nc.sync.dma_start(out=wv_sb[:, n, :], in_=wv_r[:, n, :])

                    for g in range(st // 2):
                        xr2s = x2_pending.pop(g)
                        pv = [
                            vps.tile([128, M], f32, name="pv", tag="vacc")
                            for _ in range(2)
                        ]
                        for n in range(dt):
                            for jj in range(2):
                                nc.tensor.matmul(
                                    pv[jj],
                                    lhsT=xr2s[n // (dt // 4)][
                                        :, n % (dt // 4), jj * 128 : (jj + 1) * 128
                                    ],
                                    rhs=wv_sb[:, n, :],
                                    start=(n == 0), stop=(n == dt - 1),
                                )
                        if g + 2 < st // 2:
                            load_x2(g + 2)
                        for jj in range(2):
                            nc.scalar.copy(out=v_sb[:, g * 2 + jj, :], in_=pv[jj])

                # ---- phase 3+4: attention + output projection ----
                with (
                    tc.tile_pool(name="woutp", bufs=1) as woutp,
                    tc.tile_pool(name="att", bufs=3) as attp,
                    tc.tile_pool(name="outT", bufs=2) as outTp,
                    tc.tile_pool(name="scps", bufs=2, space="PSUM") as scps,
                    tc.tile_pool(name="ops", bufs=3, space="PSUM") as ops,
                    tc.tile_pool(name="dps", bufs=1, space="PSUM") as dps,
                ):
                    wout_sb = woutp.tile([128, HL, d], bf16, name="wout_sb")
                    for h in range(HL):
                        nc.sync.dma_start(
                            out=wout_sb[:, h, :],
                            in_=wout.rearrange("(h p) e -> p h e", p=128)[:, h, :],
                        )

                    def finish_head(fin):
                        """Normalize a finished head: one ones-column matmul
                        partition-reduces the DVE/gpsimd-accumulated exp sums
                        into a [1,512] PSUM row, 1/denom is exp(-ln(d)) on the
                        scalar engine, and a K=1 ones-row matmul broadcasts it
                        across partitions (same PSUM bank); then copy + mul on
                        the vector engine.  Deferred so the reduce matmul
                        (which waits on the esum chain) sits behind the next
                        head's score matmuls in the PE queue."""
                        psum_o, esum, outT_slice = fin
                        den = dps.tile([128, 512], f32, name="den", tag="den")
                        psum_d = den[0:1, :]
                        nc.tensor.matmul(
                            psum_d, lhsT=ones_bf[:, 0:1], rhs=esum,
                            start=True, stop=True,
                        )
                        drow = attp.tile([1, 512], f32, name="drow", bufs=2)
                        nc.scalar.activation(out=drow, in_=psum_d, func=Act.Ln)
                        rrow = attp.tile([1, 512], bf16, name="rrow", bufs=2)
                        nc.scalar.activation(
                            out=rrow, in_=drow, func=Act.Exp, scale=-1.0
                        )
                        nc.tensor.matmul(
                            den, lhsT=ones_bf[0:1, :], rhs=rrow,
                            start=True, stop=True,
                        )
                        rbc = attp.tile([128, 512], bf16, name="rbc", bufs=2)
                        nc.vector.tensor_copy(out=rbc, in_=den)
                        nc.vector.tensor_mul(
                            out=outT_slice, in0=psum_o, in1=rbc
                        )

                    def emit_group(outT_p, pchunk, jj, dc):
                        """One output-projection psum group (4 matmuls over
                        heads) + copy-out + DMA.  Emitted interleaved inside
                        the next s-chunk's attention so the PE never idles
                        while the scalar engine works through the exps."""
                        srow = (pchunk * 4 + jj) * 128
                        psum_out = ops.tile(
                            [128, 512], f32, name="psum_out", tag="o"
                        )
                        for h in range(HL):
                            nc.tensor.matmul(
                                psum_out,
                                lhsT=outT_p[:, h, jj * 128 : (jj + 1) * 128],
                                rhs=wout_sb[:, h, dc * 512 : (dc + 1) * 512],
                                start=(h == 0), stop=(h == HL - 1),
                            )
                        out_sb = attp.tile([128, 512], f32, name="out_sb", bufs=4)
                        if dc == 0:
                            nc.scalar.copy(out=out_sb, in_=psum_out)
                        else:
                            nc.vector.tensor_copy(out=out_sb, in_=psum_out)
                        nc.sync.dma_start(
                            out=out[srow : srow + 128, dc * 512 : (dc + 1) * 512],
                            in_=out_sb,
                        )

                    pending = []
                    opq = []
                    for nchunk in range(nsc):
                        ssl = slice(nchunk * 512, (nchunk + 1) * 512)
                        outT_n = outTp.tile([128, HL, 512], bf16, name="outT_n")
                        for h in range(HL):
                            psum_o = ops.tile([128, 512], f32, name="po", tag="o")
                            esum_a = attp.tile(
                                [128, 512], bf16, name="esum_a", tag="esa", bufs=2
                            )
                            esum_b = attp.tile(
                                [128, 512], bf16, name="esum_b", tag="esb", bufs=2
                            )
                            for tp in range(tt // 2):
                                psc = scps.tile(
                                    [128, 1024], f32, name="psc", tag="sc"
                                )
                                for half in range(2):
                                    t = 2 * tp + half
                                    nc.tensor.matmul(
                                        psc[:, half * 512 : (half + 1) * 512],
                                        lhsT=kT[:, h, t * 128 : (t + 1) * 128],
                                        rhs=qT[:, h, ssl],
                                        start=True, stop=True,
                                    )
                                if tp == 1 and pending:
                                    finish_head(pending.pop())
                                e2 = attp.tile([128, 1024], bf16, name="e2", bufs=4)
                                nc.scalar.activation(
                                    out=e2, in_=psc, func=Act.Exp,
                                    scale=INV_SQRT_C,
                                )
                                for half in range(2):
                                    t = 2 * tp + half
                                    esl = e2[:, half * 512 : (half + 1) * 512]
                                    nc.tensor.matmul(
                                        psum_o,
                                        lhsT=v_sb[:, t, h * C : (h + 1) * C],
                                        rhs=esl,
                                        start=(t == 0), stop=(t == tt - 1),
                                    )
                                # softmax denominator: two running chains
                                # (vector + gpsimd) to split the bandwidth
                                eng = nc.vector if tp % 2 == 0 else nc.gpsimd
                                esum = esum_a if tp % 2 == 0 else esum_b
                                if tp < 2:
                                    eng.tensor_add(
                                        out=esum, in0=e2[:, 0:512],
                                        in1=e2[:, 512:1024],
                                    )
                                else:
                                    eng.tensor_add(
                                        out=esum, in0=esum, in1=e2[:, 0:512]
                                    )
                                    eng.tensor_add(
                                        out=esum, in0=esum, in1=e2[:, 512:1024]
                                    )
                            nc.vector.tensor_add(
                                out=esum_a, in0=esum_a, in1=esum_b
                            )
                            pending.append((psum_o, esum_a, outT_n[:, h, :]))
                            for _ in range(4):
                                if opq:
                                    emit_group(*opq.pop(0))
                        while pending:
                            finish_head(pending.pop())
                        opq = [
                            (outT_n, nchunk, jj, dc)
                            for jj in range(4)
                            for dc in range(4)
                        ]
                    while opq:
                        emit_group(*opq.pop(0))
    if split_waits:
        _split_multi_waits(nc)
    return nc


_NC_CACHE = {}


def _get_nc():
    if "nc" not in _NC_CACHE:
        _NC_CACHE["nc"] = build_core_kernel()
    return _NC_CACHE["nc"]


def make_in_maps(x, rope_emb, Wq, Wk, Wv, Wout, q_scale, k_scale):
    freqs = rope_emb.reshape(S, C).astype(np.float64)
    cosf = np.cos(freqs).astype(np.float32)
    sf = np.sin(freqs)
    ssinf = np.ascontiguousarray(
        np.concatenate([-sf[:, : C // 2], sf[:, C // 2 :]], axis=1), dtype=np.float32
    )
    in_maps = []
    for c in range(NCORES):
        b, hg = c // 4, c % 4
        sl = slice(hg * M, (hg + 1) * M)
        in_maps.append(
            {
                "xT": np.ascontiguousarray(x[:, b, :].T.astype(ml_dtypes.bfloat16)),
                "wq": np.ascontiguousarray(Wq[sl, :].T.astype(ml_dtypes.bfloat16)),
                "wk": np.ascontiguousarray(Wk[sl, :].T.astype(ml_dtypes.bfloat16)),
                "wv": np.ascontiguousarray(Wv[sl, :].T.astype(ml_dtypes.bfloat16)),
                "wout": np.ascontiguousarray(Wout[:, sl].T.astype(ml_dtypes.bfloat16)),
                "cosf": cosf,
                "ssinf": ssinf,
                "qs": np.ascontiguousarray(q_scale, dtype=np.float32),
                "ks": np.ascontiguousarray(k_scale, dtype=np.float32),
            }
        )
    return in_maps


def kernel(x, rope_emb, Wq, Wk, Wv, Wout, q_scale, k_scale, **run_kwargs):
    in_maps = make_in_maps(
        np.asarray(x, np.float32),
        np.asarray(rope_emb, np.float32),
        np.asarray(Wq, np.float32),
        np.asarray(Wk, np.float32),
        np.asarray(Wv, np.float32),
        np.asarray(Wout, np.float32),
        np.asarray(q_scale, np.float32),
        np.asarray(k_scale, np.float32),
    )
    nc = _get_nc()
    res = run_bass_kernel_spmd(nc, in_maps, core_ids=list(range(NCORES)), **run_kwargs)
    out = np.zeros((S, B, D), dtype=np.float32)
    for c in range(NCORES):
        out[:, c // 4, :] += res.results[c]["out"]
    if run_kwargs.get("trace"):
        kernel.last_result = res
    return out


# revision 20
# speedup vs baseline: 1.2380x; 1.2380x over previous
"""Distributed Trainium2 kernel for nn_Attention_31370441130243.

Full-input / full-output attention layer, sharded internally over the
8 NeuronCores as (batch=2) x (head-group=4): core c handles batch c//4
and heads [4*(c%4), 4*(c%4)+4).  Each core computes its QKV projections,
per-head RMSNorm + RoPE, non-causal SDPA and a partial output projection
(its Wout column block); the host sums the 4 partials per batch.

Layout strategy per core (S=2048 seq, D=2048 model, 4 local heads, C=128):
  - Q,K projections run with transposed-x stationary tiles (lhsT = xT
    block, rhs = W.T block) producing q,k in [s, c] layout where RMSNorm
    (free-dim reduce) and RoPE (free-dim half-rotation) are cheap;
    post-processing is vectorized across all 4 heads ([128, 512] tiles).
  - q,k are PE-transposed into qT,kT [c, s] for the attention matmuls
    (4 transposes share one PSUM bank, one wide copy out).
  - Scores are computed transposed (scT = kT_blk.T @ qT = [t, s]) so the
    PV matmul needs no P transpose: out.T = v_blk.T @ exp(scT).  Scores
    for two t-chunks share a [128, 1024] PSUM tile -> one Exp per pair.
  - exp() is written as bf16; the softmax denominator is accumulated on
    the vector engine (running adds of the e2 halves), then a single
    ones[128x128] matmul per (head, s-chunk) both partition-reduces and
    broadcasts it into PSUM; normalization is reciprocal + multiply on
    the vector engine.  (This removes the 16-per-head ones-column
    denominator matmuls the PE used to execute.)
  - All matmul operands are bf16 (x, Wq/Wk/Wv/Wout, qT/kT, v, exp tiles,
    outT); accumulation stays fp32 in PSUM.  RMSNorm/RoPE math runs in
    fp32 on the psum copies before the bf16 cast.  Measured rel err vs
    the fp32 reference ~6e-3 (gate 2e-2).
  - The output projection of s-chunk k is emitted at the head-slot
    boundaries of s-chunk k+1's attention, so the PE stays dense while
    the scalar engine works through the exps (keeps the PE clock-gate
    warm: idle gaps re-throttle it to 1.2 GHz).
  - DMA issue order is tuned so the PE can start within ~2us: first Wq/Wk
    slice, then the g=0/g=1 x blocks, then remaining weight slices, then
    rope tables.
"""

import math
import sys

import ml_dtypes
import numpy as np

for _p in ("/opt/trn_rl_repo",):
    if _p not in sys.path:
        sys.path.append(_p)

import bass_rust

import concourse.bass as bass
import concourse.tile as tile
from concourse import mybir
from concourse.bass_utils import run_bass_kernel_spmd
from concourse.masks import make_identity
from concourse.vector_clock import ScopedClock

S, B, D = 2048, 2, 2048
H, C = 16, 128
HL = 4                 # heads per core
M = HL * C             # local qkv rows (512)
EPS = 1e-6
NCORES = 8
INV_SQRT_C = 1.0 / math.sqrt(C)

f32 = mybir.dt.float32
f32r = mybir.dt.float32r
bf16 = mybir.dt.bfloat16
Act = mybir.ActivationFunctionType


# ---------------------------------------------------------------------------
# This container's walrus accepts at most one sync-wait command per
# instruction; the stock TileContext exit drain carries one wait per
# outstanding proc.  Split them onto single-wait NoOps.
def _split_drain_and_barrier(self, tick_clock, wait_clock):
    nc = self.nc
    probe = nc.sync.nop(nofuse=True, hint="tile_exit_waits")
    wait_clock.add_sem_waits(probe.ins, ScopedClock({None: tick_clock.global_clock}))
    si = probe.ins.sync_info
    if si is not None and si.on_wait is not None and len(si.on_wait) > 1:
        waits = list(si.on_wait)
        si.on_wait = [waits[0]]
        for w in waits[1:]:
            n2 = nc.sync.nop(nofuse=True, hint="tile_exit_waits")
            n2.ins.sync_info = bass_rust.SyncInfo(on_wait=[w], on_update=[])
    nc.sync.drain(fusable=False)
    nc.all_engine_barrier()
    popped = nc._tile_sem_poison_stack.pop()
    assert popped is self._sem_poison
    nc.clear_and_free_semaphores(list(self.sems.allocated().values()))
    nc.all_engine_barrier()


tile.TileContext._drain_and_barrier = _split_drain_and_barrier


def _split_multi_waits(nc):
    """Walrus here accepts one sync-wait per instruction; hoist extras onto
    single-wait NoOps on the same engine immediately before the instruction."""
    for f in nc.m.functions:
        for bb in f.blocks:
            out = []
            changed = False
            for inst in bb.instructions:
                si = inst.sync_info
                if si is not None and si.on_wait is not None and len(si.on_wait) > 1:
                    waits = list(si.on_wait)
                    si.on_wait = [waits[-1]]
                    for w in waits[:-1]:
                        nop = mybir.InstNoOp(
                            name=f"I-{nc.next_id()}",
                            engine=inst.engine,
                            sync_info=mybir.SyncInfo(on_wait=[w], on_update=[]),
                            bass_nofuse=True,
                        )
                        out.append(nop)
                    changed = True
                out.append(inst)
            if changed:
                bb.instructions[:] = out


def _bcast_heads(ap_2d, heads):
    """View a [128, C] AP as [128, heads, C] with a zero-stride head dim."""
    return bass.AP(
        tensor=ap_2d.tensor,
        offset=ap_2d.offset,
        ap=[ap_2d.ap[0], [0, heads], ap_2d.ap[1]],
    )


def build_core_kernel(s=S, d=D, split_waits=True):
    """One core's kernel: partial attention output for 4 heads of 1 batch."""
    st, dt, nsc, tt = s // 128, d // 128, s // 512, s // 128
    nc = bass.Bass()

    xT = nc.declare_dram_parameter("xT", [d, s], bf16, isOutput=False)
    wq = nc.declare_dram_parameter("wq", [d, M], bf16, isOutput=False)
    wk = nc.declare_dram_parameter("wk", [d, M], bf16, isOutput=False)
    wv = nc.declare_dram_parameter("wv", [d, M], bf16, isOutput=False)
    wout = nc.declare_dram_parameter("wout", [M, d], bf16, isOutput=False)
    cosf = nc.declare_dram_parameter("cosf", [s, C], f32, isOutput=False)
    ssinf = nc.declare_dram_parameter("ssinf", [s, C], f32, isOutput=False)
    qs = nc.declare_dram_parameter("qs", [C], f32, isOutput=False)
    ks = nc.declare_dram_parameter("ks", [C], f32, isOutput=False)
    out = nc.declare_dram_parameter("out", [s, d], f32, isOutput=True)

    xT_r = xT.rearrange("(n p) t -> p n t", p=128)

    with tile.TileContext(nc) as tc:
        with (
            tc.tile_pool(name="const", bufs=1) as constp,
            tc.tile_pool(name="qkt", bufs=1) as qktp,
            tc.tile_pool(name="wvp", bufs=1) as wvp,
            tc.tile_pool(name="ph2", bufs=2) as ph2,
        ):
            qT = qktp.tile([128, HL, s], bf16, name="qT")
            kT = qktp.tile([128, HL, s], bf16, name="kT")
            wv_sb = wvp.tile([128, dt, M], bf16, name="wv_sb")
            wv_r = wv.rearrange("(n p) m -> p n m", p=128)

            # ---- phase 1: Q,K proj + rmsnorm + rope + transpose ----
            with tc.tile_pool(name="rope", bufs=1) as ropep:
                cos_t = ropep.tile([128, st, C], f32, name="cos_t")
                ssin_t = ropep.tile([128, st, C], f32, name="ssin_t")
                qs_bc = constp.tile([128, C], f32, name="qs_bc")
                ks_bc = constp.tile([128, C], f32, name="ks_bc")

                with (
                    tc.tile_pool(name="wqk", bufs=1) as wqkp,
                    tc.tile_pool(name="ph1", bufs=2) as ph1,
                    tc.tile_pool(name="accps", bufs=6, space="PSUM") as accps,
                    tc.tile_pool(name="tps", bufs=2, space="PSUM") as tps,
                ):
                    # DMA issue order matters: the sync HWDGE ring is FIFO,
                    # so put what the PE needs first at the head.
                    wq_sb = wqkp.tile([128, dt, M], bf16, name="wq_sb")
                    wk_sb = wqkp.tile([128, dt, M], bf16, name="wk_sb")
                    wq_r = wq.rearrange("(n p) m -> p n m", p=128)
                    wk_r = wk.rearrange("(n p) m -> p n m", p=128)

                    xr_pending = {}

                    def load_xr(g):
                        xrows = []
                        for dh in range(4):
                            xr = ph1.tile(
                                [128, dt // 4, 256], bf16, name="xr",
                                tag="xr", bufs=8,
                            )
                            nc.sync.dma_start(
                                out=xr,
                                in_=xT_r[
                                    :, dh * (dt // 4) : (dh + 1) * (dt // 4),
                                    g * 256 : (g + 1) * 256,
                                ],
                            )
                            xrows.append(xr)
                        xr_pending[g] = xrows

                    nc.sync.dma_start(out=wq_sb[:, 0, :], in_=wq_r[:, 0, :])
                    nc.sync.dma_start(out=wk_sb[:, 0, :], in_=wk_r[:, 0, :])
                    load_xr(0)
                    for n in range(1, 8):
                        nc.sync.dma_start(out=wq_sb[:, n, :], in_=wq_r[:, n, :])
                        nc.sync.dma_start(out=wk_sb[:, n, :], in_=wk_r[:, n, :])
                    load_xr(1)
                    for n in range(8, dt):
                        nc.sync.dma_start(out=wq_sb[:, n, :], in_=wq_r[:, n, :])
                        nc.sync.dma_start(out=wk_sb[:, n, :], in_=wk_r[:, n, :])

                    # scales, rope tables (needed only by post-processing,
                    # which trails the matmul stream by most of a g-group)
                    for w_bc, w_dram in ((qs_bc, qs), (ks_bc, ks)):
                        src = bass.AP(
                            tensor=w_dram.ap().tensor, offset=0,
                            ap=[[0, 128], [1, C]],
                        )
                        nc.sync.dma_start(out=w_bc, in_=src)
                    nc.sync.dma_start(
                        out=cos_t, in_=cosf.rearrange("(n p) c -> p n c", p=128)
                    )
                    nc.sync.dma_start(
                        out=ssin_t, in_=ssinf.rearrange("(n p) c -> p n c", p=128)
                    )

                    # constants: identity + all-ones (f32 build, f32r copies;
                    # memset rejects f32r operands in this walrus), eps,
                    # rotated scale copies for the rope sin term.
                    scratch_f = constp.tile([128, 128], f32, name="scratch_f")
                    make_identity(nc, scratch_f)
                    ident = constp.tile([128, 128], bf16, name="ident")
                    nc.vector.tensor_copy(out=ident, in_=scratch_f)
                    ones_f = constp.tile([128, 128], f32, name="ones_f")
                    nc.vector.memset(ones_f, 1.0)
                    ones_bf = constp.tile([128, 128], bf16, name="ones_bf")
                    nc.vector.tensor_copy(out=ones_bf, in_=ones_f)
                    eps_t = constp.tile([128, 1], f32, name="eps_t")
                    nc.vector.memset(eps_t, EPS)

                    qs_rot = constp.tile([128, C], f32, name="qs_rot")
                    ks_rot = constp.tile([128, C], f32, name="ks_rot")
                    for w_rot, w_bc in ((qs_rot, qs_bc), (ks_rot, ks_bc)):
                        nc.gpsimd.tensor_copy(
                            out=w_rot[:, 0 : C // 2], in_=w_bc[:, C // 2 : C]
                        )
                        nc.gpsimd.tensor_copy(
                            out=w_rot[:, C // 2 : C], in_=w_bc[:, 0 : C // 2]
                        )

                    for g in range(st // 2):
                        xrows = xr_pending.pop(g)
                        pacc = []
                        for jj in range(2):
                            pq = accps.tile([128, M], f32, name="pq", tag="acc")
                            pk = accps.tile([128, M], f32, name="pk", tag="acc")
                            pacc.append((pq, pk))
                        for n in range(dt):
                            for jj in range(2):
                                xsl = xrows[n // (dt // 4)][
                                    :, n % (dt // 4), jj * 128 : (jj + 1) * 128
                                ]
                                nc.tensor.matmul(
                                    pacc[jj][0], lhsT=xsl, rhs=wq_sb[:, n, :],
                                    start=(n == 0), stop=(n == dt - 1),
                                )
                                nc.tensor.matmul(
                                    pacc[jj][1], lhsT=xsl, rhs=wk_sb[:, n, :],
                                    start=(n == 0), stop=(n == dt - 1),
                                )
                        if g + 2 < st // 2:
                            load_xr(g + 2)
                        # free the psum accumulators as fast as possible: all
                        # four copies first, then the per-tile post-processing
                        xsbs = {}
                        for jj in range(2):
                            for qk in range(2):
                                xsb = ph1.tile([128, M], f32, name="xsb", bufs=5)
                                nc.scalar.copy(out=xsb, in_=pacc[jj][qk])
                                xsbs[(jj, qk)] = xsb
                        for jj in range(2):
                            j = g * 2 + jj
                            for qk, (w_bc, w_rot, dstT) in enumerate(
                                ((qs_bc, qs_rot, qT), (ks_bc, ks_rot, kT))
                            ):
                                xsb = xsbs[(jj, qk)]
                                # rmsnorm scale 1/rms from raw q (all heads):
                                # wide square + per-head free reduce
                                sqw = ph1.tile([128, M], f32, name="sqw", bufs=2)
                                nc.scalar.activation(
                                    out=sqw, in_=xsb, func=Act.Square
                                )
                                ssq4 = ph1.tile([128, HL, 1], f32, name="ssq4", bufs=3)
                                nc.vector.tensor_reduce(
                                    out=ssq4,
                                    in_=sqw.rearrange("p (a c) -> p a c", a=HL),
                                    op=mybir.AluOpType.add,
                                    axis=mybir.AxisListType.X,
                                )
                                rms4 = ph1.tile([128, HL], f32, name="rms4", bufs=3)
                                nc.scalar.activation(
                                    out=rms4,
                                    in_=ssq4.rearrange("p a one -> p (a one)"),
                                    func=Act.Sqrt, scale=1.0 / C, bias=eps_t,
                                )
                                r4 = ph1.tile([128, HL], f32, name="r4", bufs=3)
                                nc.vector.reciprocal(out=r4, in_=rms4)
                                # per-j rope tables folded with the channel
                                # scale (gpsimd, off the critical engines)
                                cw = ph1.tile([128, C], f32, name="cw", bufs=2)
                                nc.gpsimd.tensor_mul(
                                    out=cw, in0=cos_t[:, j, :], in1=w_bc
                                )
                                sw = ph1.tile([128, C], f32, name="sw", bufs=2)
                                nc.gpsimd.tensor_mul(
                                    out=sw, in0=ssin_t[:, j, :], in1=w_rot
                                )
                                # rotate_half into sh (wide, 3-d strided)
                                sh = ph1.tile([128, HL, C], f32, name="sh", bufs=3)
                                xsb3 = xsb.rearrange("p (a c) -> p a c", a=HL)
                                nc.vector.tensor_copy(
                                    out=sh[:, :, 0 : C // 2],
                                    in_=xsb3[:, :, C // 2 : C],
                                )
                                nc.vector.tensor_copy(
                                    out=sh[:, :, C // 2 : C],
                                    in_=xsb3[:, :, 0 : C // 2],
                                )
                                t1 = ph1.tile([128, HL, C], f32, name="t1", bufs=3)
                                nc.vector.tensor_mul(
                                    out=t1, in0=xsb3, in1=_bcast_heads(cw, HL)
                                )
                                nc.vector.tensor_mul(
                                    out=sh, in0=sh, in1=_bcast_heads(sw, HL)
                                )
                                nc.vector.tensor_add(out=t1, in0=t1, in1=sh)
                                # 1/rms per head via scalar-engine copy-with-
                                # scale (casts to bf16), then PE transpose
                                xrot = ph1.tile([128, M], bf16, name="xrot", bufs=3)
                                for h in range(HL):
                                    nc.scalar.activation(
                                        out=xrot[:, h * C : (h + 1) * C],
                                        in_=t1[:, h, :],
                                        func=Act.Copy,
                                        scale=r4[:, h : h + 1],
                                    )
                                pt4 = tps.tile([128, M], bf16, name="pt4")
                                for h in range(HL):
                                    nc.tensor.transpose(
                                        pt4[:, h * C : (h + 1) * C],
                                        xrot[:, h * C : (h + 1) * C],
                                        ident,
                                    )
                                nc.scalar.copy(
                                    out=dstT[:, :, j * 128 : (j + 1) * 128],
                                    in_=pt4.rearrange("p (a c) -> p a c", a=HL),
                                )

            # ---- phase 2: V projection ----
            with tc.tile_pool(name="vpool", bufs=1) as vpool:
                v_sb = vpool.tile([128, tt, M], bf16, name="v_sb")
                outT0 = vpool.tile([128, HL, 512], bf16, name="outT0")
                with (
                    tc.tile_pool(name="p2att", bufs=1) as p2att,
                    tc.tile_pool(name="vps", bufs=2, space="PSUM") as vps,
                    tc.tile_pool(name="scp2", bufs=1, space="PSUM") as scp2,
                    tc.tile_pool(name="po4", bufs=1, space="PSUM") as po4,
                ):
                    x2_pending = {}

                    def load_x2(g):
                        xr2s = []
                        for dh in range(4):
                            x2 = ph2.tile(
                                [128, dt // 4, 256], bf16, name="x2",
                                tag="x2", bufs=8,
                            )
                            nc.sync.dma_start(
                                out=x2,
                                in_=xT_r[
                                    :, dh * (dt // 4) : (dh + 1) * (dt // 4),
                                    g * 256 : (g + 1) * 256,
                                ],
                            )
                            xr2s.append(x2)
                        x2_pending[g] = xr2s

                    nc.sync.dma_start(out=wv_sb[:, 0, :], in_=wv_r[:, 0, :])
                    load_x2(0)
                    for n in range(1, 8):
                        nc.sync.dma_start(out=wv_sb[:, n, :], in_=wv_r[:, n, :])
                    load_x2(1)
                    for n in range(8, dt):
                        nc.sync.dma_start(out=wv_sb[:, n, :], in_=wv_r[:, n, :])

                    # s-chunk 0's attention rides inside the V loop: at
                    # iteration g the V matmuls for chunks 2g,2g+1 are split
                    # into four 8-matmul blocks used as PE filler over the
                    # exp latencies of attention tp=g-1 (which consumes the v
                    # chunks copied out at the end of iteration g-1).
                    po0 = [
                        po4.tile([128, 512], f32, name=f"po0_{h}")
                        for h in range(HL)
                    ]
                    esum0 = [
                        p2att.tile([128, 512], bf16, name=f"esum0_{h}")
                        for h in range(HL)
                    ]

                    for g in range(st // 2):
                        xr2s = x2_pending.pop(g)
                        pv = [
                            vps.tile([128, M], f32, name="pv", tag="vacc")
                            for _ in range(2)
                        ]

                        def vblock(k):
                            for n in range(4 * k, 4 * k + 4):
                                for jj in range(2):
                                    nc.tensor.matmul(
                                        pv[jj],
                                        lhsT=xr2s[n // (dt // 4)][
                                            :, n % (dt // 4),
                                            jj * 128 : (jj + 1) * 128,
                                        ],
                                        rhs=wv_sb[:, n, :],
                                        start=(n == 0), stop=(n == dt - 1),
                                    )

                        if g == 0:
                            for k in range(4):
                                vblock(k)
                        else:
                            tp = g - 1
                            for h in range(HL):
                                psc = scp2.tile(
                                    [128, 1024], f32, name="psc0", tag="sc0"
                                )
                                for half in range(2):
                                    t = 2 * tp + half
                                    nc.tensor.matmul(
                                        psc[:, half * 512 : (half + 1) * 512],
                                        lhsT=kT[:, h, t * 128 : (t + 1) * 128],
                                        rhs=qT[:, h, 0:512],
                                        start=True, stop=True,
                                    )
                                vblock(h)
                                e2 = p2att.tile(
                                    [128, 1024], bf16, name="e20",
                                    tag="e20", bufs=4,
                                )
                                nc.scalar.activation(
                                    out=e2, in_=psc, func=Act.Exp,
                                    scale=INV_SQRT_C,
                                )
                                for half in range(2):
                                    t = 2 * tp + half
                                    nc.tensor.matmul(
                                        po0[h],
                                        lhsT=v_sb[:, t, h * C : (h + 1) * C],
                                        rhs=e2[:, half * 512 : (half + 1) * 512],
                                        start=(t == 0), stop=(t == tt - 1),
                                    )
                                eng = nc.vector if h % 2 == 0 else nc.gpsimd
                                if tp == 0:
                                    eng.tensor_add(
                                        out=esum0[h], in0=e2[:, 0:512],
                                        in1=e2[:, 512:1024],
                                    )
                                else:
                                    eng.tensor_add(
                                        out=esum0[h], in0=esum0[h],
                                        in1=e2[:, 0:512],
                                    )
                                    eng.tensor_add(
                                        out=esum0[h], in0=esum0[h],
                                        in1=e2[:, 512:1024],
                                    )
                        if g + 2 < st // 2:
                            load_x2(g + 2)
                        for jj in range(2):
                            nc.vector.tensor_copy(
                                out=v_sb[:, g * 2 + jj, :], in_=pv[jj]
                            )

                    # epilogue: tp=7 of s-chunk 0, then its four finishes
                    tp = tt // 2 - 1
                    for h in range(HL):
                        psc = scp2.tile([128, 1024], f32, name="psc0", tag="sc0")
                        for half in range(2):
                            t = 2 * tp + half
                            nc.tensor.matmul(
                                psc[:, half * 512 : (half + 1) * 512],
                                lhsT=kT[:, h, t * 128 : (t + 1) * 128],
                                rhs=qT[:, h, 0:512],
                                start=True, stop=True,
                            )
                        e2 = p2att.tile(
                            [128, 1024], bf16, name="e20", tag="e20", bufs=4
                        )
                        nc.scalar.activation(
                            out=e2, in_=psc, func=Act.Exp, scale=INV_SQRT_C
                        )
                        for half in range(2):
                            t = 2 * tp + half
                            nc.tensor.matmul(
                                po0[h],
                                lhsT=v_sb[:, t, h * C : (h + 1) * C],
                                rhs=e2[:, half * 512 : (half + 1) * 512],
                                start=(t == 0), stop=(t == tt - 1),
                            )
                        eng = nc.vector if h % 2 == 0 else nc.gpsimd
                        eng.tensor_add(
                            out=esum0[h], in0=esum0[h], in1=e2[:, 0:512]
                        )
                        eng.tensor_add(
                            out=esum0[h], in0=esum0[h], in1=e2[:, 512:1024]
                        )
                    for h in range(HL):
                        den = vps.tile([128, M], f32, name="pv", tag="vacc")
                        psum_d = den[0:1, :]
                        nc.tensor.matmul(
                            psum_d, lhsT=ones_bf[:, 0:1], rhs=esum0[h],
                            start=True, stop=True,
                        )
                        drow = p2att.tile([1, 512], f32, name="drow0", bufs=2)
                        nc.scalar.activation(out=drow, in_=psum_d, func=Act.Ln)
                        rrow = p2att.tile([1, 512], bf16, name="rrow0", bufs=2)
                        nc.scalar.activation(
                            out=rrow, in_=drow, func=Act.Exp, scale=-1.0
                        )
                        nc.tensor.matmul(
                            den, lhsT=ones_bf[0:1, :], rhs=rrow,
                            start=True, stop=True,
                        )
                        rbc = p2att.tile([128, 512], bf16, name="rbc0", bufs=2)
                        nc.vector.tensor_copy(out=rbc, in_=den)
                        nc.vector.tensor_mul(
                            out=outT0[:, h, :], in0=po0[h], in1=rbc
                        )

                # ---- phase 3+4: attention + output projection ----
                with (
                    tc.tile_pool(name="woutp", bufs=1) as woutp,
                    tc.tile_pool(name="att", bufs=3) as attp,
                    tc.tile_pool(name="outT", bufs=2) as outTp,
                    tc.tile_pool(name="scps", bufs=2, space="PSUM") as scps,
                    tc.tile_pool(name="ops", bufs=3, space="PSUM") as ops,
                    tc.tile_pool(name="dps", bufs=1, space="PSUM") as dps,
                ):
                    wout_sb = woutp.tile([128, HL, d], bf16, name="wout_sb")
                    for h in range(HL):
                        nc.sync.dma_start(
                            out=wout_sb[:, h, :],
                            in_=wout.rearrange("(h p) e -> p h e", p=128)[:, h, :],
                        )

                    def finish_head(fin):
                        """Normalize a finished head: one ones-column matmul
                        partition-reduces the DVE/gpsimd-accumulated exp sums
                        into a [1,512] PSUM row, 1/denom is exp(-ln(d)) on the
                        scalar engine, and a K=1 ones-row matmul broadcasts it
                        across partitions (same PSUM bank); then copy + mul on
                        the vector engine.  Deferred so the reduce matmul
                        (which waits on the esum chain) sits behind the next
                        head's score matmuls in the PE queue."""
                        psum_o, esum, outT_slice = fin
                        den = dps.tile([128, 512], f32, name="den", tag="den")
                        psum_d = den[0:1, :]
                        nc.tensor.matmul(
                            psum_d, lhsT=ones_bf[:, 0:1], rhs=esum,
                            start=True, stop=True,
                        )
                        drow = attp.tile([1, 512], f32, name="drow", bufs=2)
                        nc.scalar.activation(out=drow, in_=psum_d, func=Act.Ln)
                        rrow = attp.tile([1, 512], bf16, name="rrow", bufs=2)
                        nc.scalar.activation(
                            out=rrow, in_=drow, func=Act.Exp, scale=-1.0
                        )
                        nc.tensor.matmul(
                            den, lhsT=ones_bf[0:1, :], rhs=rrow,
                            start=True, stop=True,
                        )
                        rbc = attp.tile([128, 512], bf16, name="rbc", bufs=2)
                        nc.vector.tensor_copy(out=rbc, in_=den)
                        nc.vector.tensor_mul(
                            out=outT_slice, in0=psum_o, in1=rbc
                        )

                    def emit_group(outT_p, pchunk, jj, dc):
                        """One output-projection psum group (4 matmuls over
                        heads) + copy-out + DMA.  Emitted interleaved inside
                        the next s-chunk's attention so the PE never idles
                        while the scalar engine works through the exps."""
                        srow = (pchunk * 4 + jj) * 128
                        psum_out = ops.tile(
                            [128, 512], f32, name="psum_out", tag="o"
                        )
                        for h in range(HL):
                            nc.tensor.matmul(
                                psum_out,
                                lhsT=outT_p[:, h, jj * 128 : (jj + 1) * 128],
                                rhs=wout_sb[:, h, dc * 512 : (dc + 1) * 512],
                                start=(h == 0), stop=(h == HL - 1),
                            )
                        out_sb = attp.tile([128, 512], f32, name="out_sb", bufs=4)
                        if dc == 0:
                            nc.scalar.copy(out=out_sb, in_=psum_out)
                        else:
                            nc.vector.tensor_copy(out=out_sb, in_=psum_out)
                        nc.sync.dma_start(
                            out=out[srow : srow + 128, dc * 512 : (dc + 1) * 512],
                            in_=out_sb,
                        )

                    pending = []
                    opq = [
                        (outT0, 0, jj, dc)
                        for jj in range(4)
                        for dc in range(4)
                    ]
                    for nchunk in range(1, nsc):
                        ssl = slice(nchunk * 512, (nchunk + 1) * 512)
                        outT_n = outTp.tile([128, HL, 512], bf16, name="outT_n")
                        for h in range(HL):
                            psum_o = ops.tile([128, 512], f32, name="po", tag="o")
                            esum_a = attp.tile(
                                [128, 512], bf16, name="esum_a", tag="esa", bufs=2
                            )
                            esum_b = attp.tile(
                                [128, 512], bf16, name="esum_b", tag="esb", bufs=2
                            )
                            for tp in range(tt // 2):
                                psc = scps.tile(
                                    [128, 1024], f32, name="psc", tag="sc"
                                )
                                for half in range(2):
                                    t = 2 * tp + half
                                    nc.tensor.matmul(
                                        psc[:, half * 512 : (half + 1) * 512],
                                        lhsT=kT[:, h, t * 128 : (t + 1) * 128],
                                        rhs=qT[:, h, ssl],
                                        start=True, stop=True,
                                    )
                                if tp == 1 and pending:
                                    finish_head(pending.pop())
                                e2 = attp.tile([128, 1024], bf16, name="e2", bufs=4)
                                nc.scalar.activation(
                                    out=e2, in_=psc, func=Act.Exp,
                                    scale=INV_SQRT_C,
                                )
                                for half in range(2):
                                    t = 2 * tp + half
                                    esl = e2[:, half * 512 : (half + 1) * 512]
                                    nc.tensor.matmul(
                                        psum_o,
                                        lhsT=v_sb[:, t, h * C : (h + 1) * C],
                                        rhs=esl,
                                        start=(t == 0), stop=(t == tt - 1),
                                    )
                                # softmax denominator: two running chains
                                # (vector + gpsimd) to split the bandwidth
                                eng = nc.vector if tp % 2 == 0 else nc.gpsimd
                                esum = esum_a if tp % 2 == 0 else esum_b
                                if tp < 2:
                                    eng.tensor_add(
                                        out=esum, in0=e2[:, 0:512],
                                        in1=e2[:, 512:1024],
                                    )
                                else:
                                    eng.tensor_add(
                                        out=esum, in0=esum, in1=e2[:, 0:512]
                                    )
                                    eng.tensor_add(
                                        out=esum, in0=esum, in1=e2[:, 512:1024]
                                    )
                            nc.vector.tensor_add(
                                out=esum_a, in0=esum_a, in1=esum_b
                            )
                            pending.append((psum_o, esum_a, outT_n[:, h, :]))
                            for _ in range(4):
                                if opq:
                                    emit_group(*opq.pop(0))
                        while pending:
                            finish_head(pending.pop())
                        opq = [
                            (outT_n, nchunk, jj, dc)
                            for jj in range(4)
                            for dc in range(4)
                        ]
                    while opq:
                        emit_group(*opq.pop(0))
    if split_waits:
        _split_multi_waits(nc)
    return nc


_NC_CACHE = {}


def _get_nc():
    if "nc" not in _NC_CACHE:
        _NC_CACHE["nc"] = build_core_kernel()
    return _NC_CACHE["nc"]


def make_in_maps(x, rope_emb, Wq, Wk, Wv, Wout, q_scale, k_scale):
    freqs = rope_emb.reshape(S, C).astype(np.float64)
    cosf = np.cos(freqs).astype(np.float32)
    sf = np.sin(freqs)
    ssinf = np.ascontiguousarray(
        np.concatenate([-sf[:, : C // 2], sf[:, C // 2 :]], axis=1), dtype=np.float32
    )
    in_maps = []
    for c in range(NCORES):
        b, hg = c // 4, c % 4
        sl = slice(hg * M, (hg + 1) * M)
        in_maps.append(
            {
                "xT": np.ascontiguousarray(x[:, b, :].T.astype(ml_dtypes.bfloat16)),
                "wq": np.ascontiguousarray(Wq[sl, :].T.astype(ml_dtypes.bfloat16)),
                "wk": np.ascontiguousarray(Wk[sl, :].T.astype(ml_dtypes.bfloat16)),
                "wv": np.ascontiguousarray(Wv[sl, :].T.astype(ml_dtypes.bfloat16)),
                "wout": np.ascontiguousarray(Wout[:, sl].T.astype(ml_dtypes.bfloat16)),
                "cosf": cosf,
                "ssinf": ssinf,
                "qs": np.ascontiguousarray(q_scale, dtype=np.float32),
                "ks": np.ascontiguousarray(k_scale, dtype=np.float32),
            }
        )
    return in_maps


def kernel(x, rope_emb, Wq, Wk, Wv, Wout, q_scale, k_scale, **run_kwargs):
    in_maps = make_in_maps(
        np.asarray(x, np.float32),
        np.asarray(rope_emb, np.float32),
        np.asarray(Wq, np.float32),
        np.asarray(Wk, np.float32),
        np.asarray(Wv, np.float32),
        np.asarray(Wout, np.float32),
        np.asarray(q_scale, np.float32),
        np.asarray(k_scale, np.float32),
    )
    nc = _get_nc()
    res = run_bass_kernel_spmd(nc, in_maps, core_ids=list(range(NCORES)), **run_kwargs)
    out = np.zeros((S, B, D), dtype=np.float32)
    for c in range(NCORES):
        out[:, c // 4, :] += res.results[c]["out"]
    if run_kwargs.get("trace"):
        kernel.last_result = res
    return out
